# revision 23
# baseline (speedup 1.0000x reference)
"""Trainium2 Bass kernel for the topk-masking attention module.

Computation (per sample n):
    cams[k, hw] = relu(sum_c x[n, c, hw] * w[k, c])          # 1x1 conv, K=4
    thr[k]      = gama * max_hw(cams[k, :])
    dropped     = where(cams > thr, 0, cams)
    mean[hw]    = sum_k dropped[k, hw] / 4
    out[n,c,hw] = x[n,c,hw] * mean[hw]

Strategy: data-parallel over batch N=32 across 8 NeuronCores (4 samples
per core).  The kernel is purely DMA-bound (read x once, write out once;
compute is tiny), so all device I/O is float16: x is cast to fp16 on the
host before upload, the kernel computes with f32 accumulation (PE PSUM,
threshold compare in f32) and stores fp16 outputs which the host casts
back to f32.  That halves both DMA directions vs f32 while keeping the
global rel err ~7e-3 (fp16 product noise is small enough that the
gama*max threshold compare flips essentially no mask bits).

Per sample, x[n] ([4096, 784] fp16, 6.4 MB) is loaded into SBUF once as
8 piece tiles [128, 4, 784] and kept resident: the 1x1 conv runs as 32
accumulating fp16 matmuls (contraction 4096 = 32x128 on partitions)
into PSUM [4, 784] f32; the per-channel max / threshold / mask run on
ACT+DVE in f32; the channel-mean + broadcast to 128 partitions is a
single matmul with a constant [4, 128] lhsT of 0.25, converted to an
fp16 SBUF tile on ACT; and the final elementwise multiply (all-fp16 on
DVE, 2x throughput) reuses the resident x tiles in-place before the
store.  HBM traffic is the 2x fp16 floor.
"""

import hashlib
import os
import sys

for _p in ("/opt/trn_rl_repo",):
    if _p not in sys.path:
        sys.path.insert(0, _p)

import numpy as np

N_CORES = 8
NFULL = 32            # full batch
NS = NFULL // N_CORES  # samples per core
C = 4096
K = 4
HW = 28 * 28          # 784
NCHUNK = C // 128     # 32
NQ = 4                # quarter tiles per sample
CPQ = NCHUNK // NQ    # 8 chunks per quarter
HALVES = ((0, 512), (512, HW))  # PSUM-bank-aligned column split

_CACHE = {}


def build_nc(n_pieces=8, x_bufs=21, cams_bufs=2, mean_bufs=2,
             store_engine="scalar", gpsimd_pieces=1, pe_filler=4,
             io_dtype="float16", prefetch=2, last_pieces=16,
             gpsimd_last=3):
    """Trace + schedule + compile the per-core Bass program.

    n_pieces: how many SBUF tiles one sample's x is split into (must
        divide 32); x_bufs slots of [128, 32/n_pieces, 784] each.
    store_engine: which engine issues output DMAs ("sync"/"scalar"/"gpsimd")
        — separate HWDGE ring from the loads avoids FIFO coupling.
    gpsimd_pieces: how many of the per-sample multiply pieces run on
        GpSimd instead of DVE (load balancing).
    io_dtype: dtype of x/w/out device tensors ("float16"/"bfloat16"/
        "float32").  Accumulation and the threshold compare stay f32.
    last_pieces/gpsimd_last: the LAST sample's multiply+store chain is the
        exposed kernel tail (nothing overlaps it), so it gets finer pieces
        and a bigger GpSimd share: multiply stream ~11 us instead of ~14.
    """
    from contextlib import ExitStack

    import concourse.bacc as bacc
    import concourse.tile as tile
    from concourse import mybir

    f32 = mybir.dt.float32
    fio = getattr(mybir.dt, io_dtype)
    nc = bacc.Bacc("TRN2", target_bir_lowering=False, debug=False,
                   num_devices=N_CORES)

    NP = n_pieces
    CPP = NCHUNK // NP  # chunks per piece
    NPL = last_pieces or NP
    CPPL = NCHUNK // NPL

    def sample_np_cpp(n):
        return (NPL, CPPL) if n == NS - 1 else (NP, CPP)

    x_d = nc.dram_tensor("x", [NS, C, HW], fio, kind="ExternalInput")
    w_d = nc.dram_tensor("w", [128, NCHUNK, K], fio, kind="ExternalInput")
    gam_d = nc.dram_tensor("gam", [K, 1], f32, kind="ExternalInput")
    qlhs_d = nc.dram_tensor("qlhs", [K, 128], fio, kind="ExternalInput")
    out_d = nc.dram_tensor("out", [NS, C, HW], fio, kind="ExternalOutput")

    # [NS, C, HW] viewed as [NS, 128(part), NCHUNK, HW]: partition p holds
    # the NCHUNK *adjacent* channels c = p*NCHUNK + j.  Each (partition,
    # piece) DMA run is then CPP*3136 contiguous bytes — fewer, longer
    # descriptors than the chunk-major c = j*128 + p mapping — and the w
    # host packing in make_in_maps is a plain reshape with the same mapping.
    x_v = x_d.ap().rearrange("n (p j) hw -> n p j hw", p=128, j=NCHUNK)
    out_v = out_d.ap().rearrange("n (p j) hw -> n p j hw", p=128, j=NCHUNK)

    if isinstance(store_engine, (list, tuple)):
        store_engs = [getattr(nc, e) for e in store_engine]
    else:
        store_engs = [getattr(nc, store_engine)]

    with tile.TileContext(nc) as tc, ExitStack() as ctx:
        consts = ctx.enter_context(tc.tile_pool(name="consts", bufs=1))
        xpool = ctx.enter_context(tc.tile_pool(name="xpool", bufs=x_bufs))
        xlpool = ctx.enter_context(tc.tile_pool(name="xlpool", bufs=NPL))
        spool = ctx.enter_context(tc.tile_pool(name="spool", bufs=2))
        cpsum = ctx.enter_context(
            tc.tile_pool(name="cpsum", bufs=cams_bufs, space="PSUM"))
        mpsum = ctx.enter_context(
            tc.tile_pool(name="mpsum", bufs=mean_bufs, space="PSUM"))

        w_sb = consts.tile([128, NCHUNK, K], fio, name="w_sb")
        nc.scalar.dma_start(w_sb[:], w_d.ap())
        gam_sb = consts.tile([K, 1], f32, name="gam_sb")
        nc.scalar.dma_start(gam_sb[:], gam_d.ap())
        qlhs_sb = consts.tile([K, 128], fio, name="qlhs_sb")
        nc.scalar.dma_start(qlhs_sb[:], qlhs_d.ap())

        xq_all = {}
        cams_all = {}

        def emit_loads(n, engines=(nc.sync,)):
            # Sample 0 may split across both HWDGE rings: the store ring
            # (qActDynamicHW) is empty until the first mean is ready, so
            # lending it to the head loads gets mean(0) (and with it the
            # whole store stream) started earlier.  Later samples must stay
            # off the store ring: a load enqueued behind a not-yet-ready
            # store would head-of-line block there.
            np_, cpp_ = sample_np_cpp(n)
            pool = xlpool if np_ == NPL and n == NS - 1 else xpool
            xq_all[n] = []
            for q in range(np_):
                t = pool.tile([128, cpp_, HW], fio, tag="xq",
                              name=f"xq_{n}_{q}")
                engines[q % len(engines)].dma_start(
                    t[:], x_v[n][:, q * cpp_:(q + 1) * cpp_, :])
                xq_all[n].append(t)

        def emit_chunk_mms(n, j_lo, j_hi):
            cams = cams_all[n]
            xq = xq_all[n]
            _, cpp_ = sample_np_cpp(n)
            for j in range(j_lo, j_hi):
                q, jj = divmod(j, cpp_)
                for c0, c1 in HALVES:
                    nc.tensor.matmul(
                        cams[:, c0:c1],
                        w_sb[:, j, :],
                        xq[q][:, jj, c0:c1],
                        start=(j == 0),
                        stop=(j == NCHUNK - 1),
                    )

        # Queue loads `prefetch` samples ahead so the HBM read stream never
        # starves while a sample's mean chain (mask -> mean -> multiply)
        # briefly stalls the consumers.  SBUF holds prefetch+1 samples of x.
        for n0 in range(min(prefetch, NS)):
            emit_loads(n0, engines=(nc.sync, nc.scalar) if n0 == 0
                       else (nc.sync,))
        for n in range(NS):
            if n not in cams_all:
                cams_all[n] = cpsum.tile([K, HW], f32, tag="cams",
                                         name=f"cams_{n}")
            emit_chunk_mms(n, pe_filler if n > 0 else 0, NCHUNK)
            cams = cams_all[n]
            xq = xq_all[n]

            # relu on ACT (PSUM -> SBUF)
            r = spool.tile([K, HW], f32, tag="r", name=f"r_{n}")
            nc.scalar.activation(r[:], cams[:],
                                 mybir.ActivationFunctionType.Relu)
            # per-channel spatial max
            rmax = spool.tile([K, 1], f32, tag="rmax", name=f"rmax_{n}")
            nc.vector.tensor_reduce(rmax[:], r[:], axis=mybir.AxisListType.X,
                                    op=mybir.AluOpType.max)
            # thr = gama * max
            thr = spool.tile([K, 1], f32, tag="thr", name=f"thr_{n}")
            nc.vector.tensor_scalar(thr[:], rmax[:], gam_sb[:], None,
                                    op0=mybir.AluOpType.mult)
            # masked = (r <= thr) * r, cast to the IO dtype for the mean
            # matmul (f32 compare, so no extra mask flips from this cast)
            masked = spool.tile([K, HW], fio, tag="masked", name=f"masked_{n}")
            nc.vector.scalar_tensor_tensor(masked[:], r[:], thr[:], r[:],
                                           op0=mybir.AluOpType.is_le,
                                           op1=mybir.AluOpType.mult)
            # Keep PE busy while the DVE mask for sample n completes:
            # emit the first pe_filler chunk matmuls of sample n+1 ahead of
            # sample n's mean matmul in PE program order (in-order engine,
            # head-of-line blocking otherwise; also avoids a HAM idle gap).
            if n + prefetch < NS:
                emit_loads(n + prefetch)
            if n + 1 < NS and pe_filler:
                cams_all[n + 1] = cpsum.tile([K, HW], f32, tag="cams",
                                             name=f"cams_{n + 1}")
                emit_chunk_mms(n + 1, 0, pe_filler)

            # mean over k, broadcast to 128 partitions: qlhs (0.25) matmul
            meanb = mpsum.tile([128, HW], f32, tag="meanb", name=f"meanb_{n}")
            for c0, c1 in HALVES:
                nc.tensor.matmul(meanb[:, c0:c1], qlhs_sb[:],
                                 masked[:, c0:c1], start=True, stop=True)

            # PSUM f32 -> SBUF fio on ACT so the final multiply is all
            # 16-bit on DVE (2x throughput) and PSUM banks free up early.
            mean_sb = spool.tile([128, HW], fio, tag="mean_sb",
                                 name=f"mean_sb_{n}")
            nc.scalar.activation(mean_sb[:], meanb[:],
                                 mybir.ActivationFunctionType.Copy)

            # For the last sample all loads are already enqueued, so the
            # load ring (qSPDynamicHW) is free to take half the tail
            # stores — both rings drain the final store backlog in
            # parallel.  (Mirror of the sample-0 load split above.)
            last = n == NS - 1
            s_engs = (store_engs + [nc.sync]) if last else store_engs
            np_, cpp_ = sample_np_cpp(n)
            gps = gpsimd_last if last else gpsimd_pieces
            mb = mean_sb.unsqueeze(1).broadcast_to([128, cpp_, HW])
            for q in range(np_):
                # GpSimd is ~3.7x slower per piece than DVE; give it the
                # LAST pieces so its slow multiplies never head-of-line
                # block the store FIFO in front of fast DVE pieces (it
                # starts at mean-ready regardless, trimming DVE's chain).
                eng = nc.gpsimd if q >= np_ - gps else nc.vector
                eng.tensor_tensor(xq[q][:], xq[q][:], mb,
                                  op=mybir.AluOpType.mult)
                s_engs[q % len(s_engs)].dma_start(
                    out_v[n][:, q * cpp_:(q + 1) * cpp_, :], xq[q][:])

    nc.compile()
    return nc


def _get_nc():
    if "nc" not in _CACHE:
        _CACHE["nc"] = build_nc()
    return _CACHE["nc"]


IO_NP_DTYPE = np.float16


def make_in_maps(x, fc_weights, gama):
    """Shard/pack full numpy inputs into per-core input maps."""
    x = np.ascontiguousarray(
        np.asarray(x, dtype=np.float32).reshape(NFULL, C, HW).astype(IO_NP_DTYPE))
    fcw = np.asarray(fc_weights, dtype=np.float32).reshape(K, C)
    # w_arr[p, j, k] = fcw[k, p*NCHUNK + j]  (channel c = p*NCHUNK + j,
    # matching the x view in build_nc)
    w_arr = np.ascontiguousarray(fcw.T.reshape(128, NCHUNK, K)).astype(IO_NP_DTYPE)
    gam4 = np.full((K, 1), np.float32(np.asarray(gama)), dtype=np.float32)
    qlhs = np.full((K, 128), 0.25, dtype=IO_NP_DTYPE)
    in_maps = []
    for c in range(N_CORES):
        in_maps.append({
            "x": x[c * NS:(c + 1) * NS],
            "w": w_arr,
            "gam": gam4,
            "qlhs": qlhs,
        })
    return in_maps


def _strip_debug(obj):
    """Recursively blank debug-only fields (file paths / tracebacks) so the
    cache key is independent of where kernel.py lives on disk."""
    if isinstance(obj, dict):
        return {
            k: ("" if k in ("filename", "ant_traceback") else _strip_debug(v))
            for k, v in obj.items()
        }
    if isinstance(obj, list):
        return [_strip_debug(v) for v in obj]
    return obj


def _bass_module_cache_key(code, code_format):
    """Semantic cache key for a bass_exec HLO module, or None.

    Hashes the embedded BIR with debug-only fields blanked, plus the
    IO-name/arch config.  Any semantic difference changes the key; a
    path-only difference (same kernel traced from another directory)
    does not.
    """
    import base64
    import json

    if b"bass_exec" not in bytes(code) or bytes(code_format) != b"hlo":
        return None
    import libneuronxla.proto.hlo_pb2 as hlo_pb2
    from concourse import bass2jax

    proto = hlo_pb2.HloModuleProto.FromString(bytes(code))
    cfgs = [
        ins.backend_config
        for comp in proto.computations
        for ins in comp.instructions
        if ins.opcode == "custom-call" and ins.custom_call_target == "bass_exec"
    ]
    if len(cfgs) != 1:
        return None
    config = json.loads(base64.standard_b64decode(cfgs[0]))
    decomp = getattr(bass2jax, "_decompress_ant_bir", None)
    if decomp is None:
        return None
    bir = json.loads(decomp(config["ant_bir"]))
    h = hashlib.sha256()
    h.update(json.dumps(_strip_debug(bir), sort_keys=True).encode())
    h.update(json.dumps(
        [config.get("in_names"), config.get("out_names"),
         config.get("arch"), proto.name],
        sort_keys=True).encode())
    return h.hexdigest()


def _install_neff_cache():
    """Wrap concourse's neuronx_cc hook with a content-keyed NEFF cache.

    The stock hook recompiles the NEFF from scratch in every process
    (minutes for this kernel); the emitted BIR is deterministic modulo
    debug file paths, so a debug-stripped content hash makes repeat
    compiles of the identical module instant.
    """
    if _CACHE.get("cc_cached"):
        return
    try:
        from concourse import bass2jax

        inner = bass2jax.neuronx_cc_hook
        cache_dir = os.path.expanduser("~/.cache/bass_neff_cache")
        os.makedirs(cache_dir, exist_ok=True)

        def cached_hook(code, code_format, platform_version, file_prefix):
            path = None
            try:
                key = _bass_module_cache_key(code, code_format)
                if key is not None:
                    path = os.path.join(cache_dir, key)
                    if os.path.exists(path):
                        with open(path, "rb") as f:
                            return 0, f.read()
            except Exception:
                path = None
            ret, data = inner(code, code_format, platform_version, file_prefix)
            if path is not None and ret == 0:
                try:
                    tmp = f"{path}.tmp{os.getpid()}"
                    with open(tmp, "wb") as f:
                        f.write(data)
                    os.replace(tmp, path)
                except Exception:
                    pass
            return ret, data

        bass2jax.neuronx_cc_hook = cached_hook
        # If the plain hook was already installed on libneuronxla, refresh it.
        try:
            import libneuronxla

            if getattr(libneuronxla, "orig_neuronx_cc", None) is not None:
                libneuronxla.neuronx_cc = cached_hook
        except ImportError:
            pass
        _CACHE["cc_cached"] = True
    except Exception:
        pass


def kernel(x, fc_weights, gama):
    from concourse.bass_utils import run_bass_kernel_spmd

    _install_neff_cache()
    nc = _get_nc()
    in_maps = make_in_maps(x, fc_weights, gama)
    res = run_bass_kernel_spmd(nc, in_maps, core_ids=list(range(N_CORES)))
    out = np.concatenate([np.asarray(r["out"]) for r in res.results], axis=0)
    return out.reshape(NFULL, C, 28, 28).astype(np.float32)



# revision 27
# speedup vs baseline: 1.4458x; 1.4458x over previous
"""Trainium2 Bass kernel for the topk-masking attention module.

Computation (per sample n):
    cams[k, hw] = relu(sum_c x[n, c, hw] * w[k, c])          # 1x1 conv, K=4
    thr[k]      = gama * max_hw(cams[k, :])
    dropped     = where(cams > thr, 0, cams)
    mean[hw]    = sum_k dropped[k, hw] / 4
    out[n,c,hw] = x[n,c,hw] * mean[hw]

Strategy: data-parallel over batch N=32 across 8 NeuronCores (4 samples
per core).  The kernel is purely DMA-bound (read x once, write out once;
compute is tiny), so all device I/O is float16: x is cast to fp16 on the
host before upload, the kernel computes with f32 accumulation (PE PSUM,
threshold compare in f32) and stores fp16 outputs which the host casts
back to f32.  That halves both DMA directions vs f32 while keeping the
global rel err ~7e-3 (fp16 product noise is small enough that the
gama*max threshold compare flips essentially no mask bits).

Per sample, x[n] ([4096, 784] fp16, 6.4 MB) is loaded into SBUF once as
8 piece tiles [128, 4, 784] and kept resident: the 1x1 conv runs as 32
accumulating fp16 matmuls (contraction 4096 = 32x128 on partitions)
into PSUM [4, 784] f32; the per-channel max / threshold / mask run on
ACT+DVE in f32; the channel-mean + broadcast to 128 partitions is a
single matmul with a constant [4, 128] lhsT of 0.25, converted to an
fp16 SBUF tile on ACT; and the final elementwise multiply (all-fp16 on
DVE, 2x throughput) reuses the resident x tiles in-place before the
store.  HBM traffic is the 2x fp16 floor.
"""

import hashlib
import os
import sys

for _p in ("/opt/trn_rl_repo",):
    if _p not in sys.path:
        sys.path.insert(0, _p)

import numpy as np

N_CORES = 8
NFULL = 32            # full batch
NS = NFULL // N_CORES  # samples per core
C = 4096
K = 4
HW = 28 * 28          # 784
NCHUNK = C // 128     # 32
NQ = 4                # quarter tiles per sample
CPQ = NCHUNK // NQ    # 8 chunks per quarter
HALVES = ((0, 512), (512, HW))  # PSUM-bank-aligned column split

_CACHE = {}


def build_nc(n_pieces=8, x_bufs=17, cams_bufs=2, mean_bufs=2,
             store_engine="scalar", gpsimd_pieces=1, pe_filler=4,
             io_dtype="float16", prefetch=2, last_pieces=16,
             gpsimd_last=3):
    """Trace + schedule + compile the per-core Bass program.

    n_pieces: how many SBUF tiles one sample's x is split into (must
        divide 32); x_bufs slots of [128, 32/n_pieces, 784] each.
    store_engine: which engine issues output DMAs ("sync"/"scalar"/"gpsimd")
        — separate HWDGE ring from the loads avoids FIFO coupling.
    gpsimd_pieces: how many of the per-sample multiply pieces run on
        GpSimd instead of DVE (load balancing).
    io_dtype: dtype of x/w/out device tensors ("float16"/"bfloat16"/
        "float32").  Accumulation and the threshold compare stay f32.
    last_pieces/gpsimd_last: the LAST sample's multiply+store chain is the
        exposed kernel tail (nothing overlaps it), so it gets finer pieces
        and a bigger GpSimd share: multiply stream ~11 us instead of ~14.
    """
    from contextlib import ExitStack

    import concourse.bacc as bacc
    import concourse.tile as tile
    from concourse import mybir

    f32 = mybir.dt.float32
    fio = getattr(mybir.dt, io_dtype)
    nc = bacc.Bacc("TRN2", target_bir_lowering=False, debug=False,
                   num_devices=N_CORES)

    NP = n_pieces
    CPP = NCHUNK // NP  # chunks per piece
    NPL = last_pieces or NP
    CPPL = NCHUNK // NPL

    def fine(n):
        # Sample 0 (head: first store gates the whole store stream) and
        # the last sample (tail: nothing overlaps it) get fine pieces.
        return n == 0 or n == NS - 1

    def sample_np_cpp(n):
        return (NPL, CPPL) if fine(n) else (NP, CPP)

    x_d = nc.dram_tensor("x", [NS, C, HW], fio, kind="ExternalInput")
    w_d = nc.dram_tensor("w", [128, NCHUNK, K], fio, kind="ExternalInput")
    gam_d = nc.dram_tensor("gam", [K, 1], f32, kind="ExternalInput")
    qlhs_d = nc.dram_tensor("qlhs", [K, 128], fio, kind="ExternalInput")
    out_d = nc.dram_tensor("out", [NS, C, HW], fio, kind="ExternalOutput")

    # [NS, C, HW] viewed as [NS, 128(part), NCHUNK, HW]: partition p holds
    # the NCHUNK *adjacent* channels c = p*NCHUNK + j.  Each (partition,
    # piece) DMA run is then CPP*3136 contiguous bytes — fewer, longer
    # descriptors than the chunk-major c = j*128 + p mapping — and the w
    # host packing in make_in_maps is a plain reshape with the same mapping.
    x_v = x_d.ap().rearrange("n (p j) hw -> n p j hw", p=128, j=NCHUNK)
    out_v = out_d.ap().rearrange("n (p j) hw -> n p j hw", p=128, j=NCHUNK)

    if isinstance(store_engine, (list, tuple)):
        store_engs = [getattr(nc, e) for e in store_engine]
    else:
        store_engs = [getattr(nc, store_engine)]

    with tile.TileContext(nc) as tc, ExitStack() as ctx:
        consts = ctx.enter_context(tc.tile_pool(name="consts", bufs=1))
        xpool = ctx.enter_context(tc.tile_pool(name="xpool", bufs=x_bufs))
        xlpool = ctx.enter_context(tc.tile_pool(name="xlpool", bufs=NPL))
        spool = ctx.enter_context(tc.tile_pool(name="spool", bufs=2))
        cpsum = ctx.enter_context(
            tc.tile_pool(name="cpsum", bufs=cams_bufs, space="PSUM"))
        mpsum = ctx.enter_context(
            tc.tile_pool(name="mpsum", bufs=mean_bufs, space="PSUM"))

        w_sb = consts.tile([128, NCHUNK, K], fio, name="w_sb")
        nc.scalar.dma_start(w_sb[:], w_d.ap())
        gam_sb = consts.tile([K, 1], f32, name="gam_sb")
        nc.scalar.dma_start(gam_sb[:], gam_d.ap())
        qlhs_sb = consts.tile([K, 128], fio, name="qlhs_sb")
        nc.scalar.dma_start(qlhs_sb[:], qlhs_d.ap())

        xq_all = {}
        cams_all = {}

        def emit_loads(n, engines=(nc.sync,)):
            # Sample 0 may split across both HWDGE rings: the store ring
            # (qActDynamicHW) is empty until the first mean is ready, so
            # lending it to the head loads gets mean(0) (and with it the
            # whole store stream) started earlier.  Later samples must stay
            # off the store ring: a load enqueued behind a not-yet-ready
            # store would head-of-line block there.
            np_, cpp_ = sample_np_cpp(n)
            # Fine samples (0 and NS-1) share xlpool slots: sample 0's
            # tiles are stored (and freed) long before the last sample's
            # loads want the slots back.
            pool = xlpool if fine(n) else xpool
            xq_all[n] = []
            for q in range(np_):
                t = pool.tile([128, cpp_, HW], fio, tag="xq",
                              name=f"xq_{n}_{q}")
                engines[q % len(engines)].dma_start(
                    t[:], x_v[n][:, q * cpp_:(q + 1) * cpp_, :])
                xq_all[n].append(t)

        def emit_chunk_mms(n, j_lo, j_hi):
            cams = cams_all[n]
            xq = xq_all[n]
            _, cpp_ = sample_np_cpp(n)
            for j in range(j_lo, j_hi):
                q, jj = divmod(j, cpp_)
                for c0, c1 in HALVES:
                    nc.tensor.matmul(
                        cams[:, c0:c1],
                        w_sb[:, j, :],
                        xq[q][:, jj, c0:c1],
                        start=(j == 0),
                        stop=(j == NCHUNK - 1),
                    )

        # Queue loads `prefetch` samples ahead so the HBM read stream never
        # starves while a sample's mean chain (mask -> mean -> multiply)
        # briefly stalls the consumers.  SBUF holds prefetch+1 samples of x.
        for n0 in range(min(prefetch, NS)):
            emit_loads(n0, engines=(nc.sync, nc.scalar) if n0 == 0
                       else (nc.sync,))
        for n in range(NS):
            if n not in cams_all:
                cams_all[n] = cpsum.tile([K, HW], f32, tag="cams",
                                         name=f"cams_{n}")
            emit_chunk_mms(n, pe_filler if n > 0 else 0, NCHUNK)
            cams = cams_all[n]
            xq = xq_all[n]

            # relu on ACT (PSUM -> SBUF)
            r = spool.tile([K, HW], f32, tag="r", name=f"r_{n}")
            nc.scalar.activation(r[:], cams[:],
                                 mybir.ActivationFunctionType.Relu)
            # per-channel spatial max
            rmax = spool.tile([K, 1], f32, tag="rmax", name=f"rmax_{n}")
            nc.vector.tensor_reduce(rmax[:], r[:], axis=mybir.AxisListType.X,
                                    op=mybir.AluOpType.max)
            # thr = gama * max
            thr = spool.tile([K, 1], f32, tag="thr", name=f"thr_{n}")
            nc.vector.tensor_scalar(thr[:], rmax[:], gam_sb[:], None,
                                    op0=mybir.AluOpType.mult)
            # masked = (r <= thr) * r, cast to the IO dtype for the mean
            # matmul (f32 compare, so no extra mask flips from this cast)
            masked = spool.tile([K, HW], fio, tag="masked", name=f"masked_{n}")
            nc.vector.scalar_tensor_tensor(masked[:], r[:], thr[:], r[:],
                                           op0=mybir.AluOpType.is_le,
                                           op1=mybir.AluOpType.mult)
            # Keep PE busy while the DVE mask for sample n completes:
            # emit the first pe_filler chunk matmuls of sample n+1 ahead of
            # sample n's mean matmul in PE program order (in-order engine,
            # head-of-line blocking otherwise; also avoids a HAM idle gap).
            if n + prefetch < NS:
                emit_loads(n + prefetch)
            if n + 1 < NS and pe_filler:
                cams_all[n + 1] = cpsum.tile([K, HW], f32, tag="cams",
                                             name=f"cams_{n + 1}")
                emit_chunk_mms(n + 1, 0, pe_filler)

            # mean over k, broadcast to 128 partitions: qlhs (0.25) matmul
            meanb = mpsum.tile([128, HW], f32, tag="meanb", name=f"meanb_{n}")
            for c0, c1 in HALVES:
                nc.tensor.matmul(meanb[:, c0:c1], qlhs_sb[:],
                                 masked[:, c0:c1], start=True, stop=True)

            # PSUM f32 -> SBUF fio on ACT so the final multiply is all
            # 16-bit on DVE (2x throughput) and PSUM banks free up early.
            mean_sb = spool.tile([128, HW], fio, tag="mean_sb",
                                 name=f"mean_sb_{n}")
            nc.scalar.activation(mean_sb[:], meanb[:],
                                 mybir.ActivationFunctionType.Copy)

            # For the last sample all loads are already enqueued, so the
            # load ring (qSPDynamicHW) is free to take half the tail
            # stores — both rings drain the final store backlog in
            # parallel.  (Mirror of the sample-0 load split above.)
            last = n == NS - 1
            s_engs = (store_engs + [nc.sync]) if last else store_engs
            np_, cpp_ = sample_np_cpp(n)
            gps = gpsimd_last if fine(n) else gpsimd_pieces
            mb = mean_sb.unsqueeze(1).broadcast_to([128, cpp_, HW])
            for q in range(np_):
                # GpSimd is ~3.7x slower per piece than DVE; give it the
                # LAST pieces so its slow multiplies never head-of-line
                # block the store FIFO in front of fast DVE pieces (it
                # starts at mean-ready regardless, trimming DVE's chain).
                eng = nc.gpsimd if q >= np_ - gps else nc.vector
                eng.tensor_tensor(xq[q][:], xq[q][:], mb,
                                  op=mybir.AluOpType.mult)
                s_engs[q % len(s_engs)].dma_start(
                    out_v[n][:, q * cpp_:(q + 1) * cpp_, :], xq[q][:])

    nc.compile()
    return nc


def _get_nc():
    if "nc" not in _CACHE:
        _CACHE["nc"] = build_nc()
    return _CACHE["nc"]


IO_NP_DTYPE = np.float16


def make_in_maps(x, fc_weights, gama):
    """Shard/pack full numpy inputs into per-core input maps."""
    x = np.ascontiguousarray(
        np.asarray(x, dtype=np.float32).reshape(NFULL, C, HW).astype(IO_NP_DTYPE))
    fcw = np.asarray(fc_weights, dtype=np.float32).reshape(K, C)
    # w_arr[p, j, k] = fcw[k, p*NCHUNK + j]  (channel c = p*NCHUNK + j,
    # matching the x view in build_nc)
    w_arr = np.ascontiguousarray(fcw.T.reshape(128, NCHUNK, K)).astype(IO_NP_DTYPE)
    gam4 = np.full((K, 1), np.float32(np.asarray(gama)), dtype=np.float32)
    qlhs = np.full((K, 128), 0.25, dtype=IO_NP_DTYPE)
    in_maps = []
    for c in range(N_CORES):
        in_maps.append({
            "x": x[c * NS:(c + 1) * NS],
            "w": w_arr,
            "gam": gam4,
            "qlhs": qlhs,
        })
    return in_maps


def _strip_debug(obj):
    """Recursively blank debug-only fields (file paths / tracebacks) so the
    cache key is independent of where kernel.py lives on disk."""
    if isinstance(obj, dict):
        return {
            k: ("" if k in ("filename", "ant_traceback") else _strip_debug(v))
            for k, v in obj.items()
        }
    if isinstance(obj, list):
        return [_strip_debug(v) for v in obj]
    return obj


def _bass_module_cache_key(code, code_format):
    """Semantic cache key for a bass_exec HLO module, or None.

    Hashes the embedded BIR with debug-only fields blanked, plus the
    IO-name/arch config.  Any semantic difference changes the key; a
    path-only difference (same kernel traced from another directory)
    does not.
    """
    import base64
    import json

    if b"bass_exec" not in bytes(code) or bytes(code_format) != b"hlo":
        return None
    import libneuronxla.proto.hlo_pb2 as hlo_pb2
    from concourse import bass2jax

    proto = hlo_pb2.HloModuleProto.FromString(bytes(code))
    cfgs = [
        ins.backend_config
        for comp in proto.computations
        for ins in comp.instructions
        if ins.opcode == "custom-call" and ins.custom_call_target == "bass_exec"
    ]
    if len(cfgs) != 1:
        return None
    config = json.loads(base64.standard_b64decode(cfgs[0]))
    decomp = getattr(bass2jax, "_decompress_ant_bir", None)
    if decomp is None:
        return None
    bir = json.loads(decomp(config["ant_bir"]))
    h = hashlib.sha256()
    h.update(json.dumps(_strip_debug(bir), sort_keys=True).encode())
    h.update(json.dumps(
        [config.get("in_names"), config.get("out_names"),
         config.get("arch"), proto.name],
        sort_keys=True).encode())
    return h.hexdigest()


def _install_neff_cache():
    """Wrap concourse's neuronx_cc hook with a content-keyed NEFF cache.

    The stock hook recompiles the NEFF from scratch in every process
    (minutes for this kernel); the emitted BIR is deterministic modulo
    debug file paths, so a debug-stripped content hash makes repeat
    compiles of the identical module instant.
    """
    if _CACHE.get("cc_cached"):
        return
    try:
        from concourse import bass2jax

        inner = bass2jax.neuronx_cc_hook
        cache_dir = os.path.expanduser("~/.cache/bass_neff_cache")
        os.makedirs(cache_dir, exist_ok=True)

        def cached_hook(code, code_format, platform_version, file_prefix):
            path = None
            try:
                key = _bass_module_cache_key(code, code_format)
                if key is not None:
                    path = os.path.join(cache_dir, key)
                    if os.path.exists(path):
                        with open(path, "rb") as f:
                            return 0, f.read()
            except Exception:
                path = None
            ret, data = inner(code, code_format, platform_version, file_prefix)
            if path is not None and ret == 0:
                try:
                    tmp = f"{path}.tmp{os.getpid()}"
                    with open(tmp, "wb") as f:
                        f.write(data)
                    os.replace(tmp, path)
                except Exception:
                    pass
            return ret, data

        bass2jax.neuronx_cc_hook = cached_hook
        # If the plain hook was already installed on libneuronxla, refresh it.
        try:
            import libneuronxla

            if getattr(libneuronxla, "orig_neuronx_cc", None) is not None:
                libneuronxla.neuronx_cc = cached_hook
        except ImportError:
            pass
        _CACHE["cc_cached"] = True
    except Exception:
        pass


def kernel(x, fc_weights, gama):
    from concourse.bass_utils import run_bass_kernel_spmd

    _install_neff_cache()
    nc = _get_nc()
    in_maps = make_in_maps(x, fc_weights, gama)
    res = run_bass_kernel_spmd(nc, in_maps, core_ids=list(range(N_CORES)))
    out = np.concatenate([np.asarray(r["out"]) for r in res.results], axis=0)
    return out.reshape(NFULL, C, 28, 28).astype(np.float32)



# revision 28
# speedup vs baseline: 1.4739x; 1.0195x over previous
"""Trainium2 Bass kernel for the topk-masking attention module.

Computation (per sample n):
    cams[k, hw] = relu(sum_c x[n, c, hw] * w[k, c])          # 1x1 conv, K=4
    thr[k]      = gama * max_hw(cams[k, :])
    dropped     = where(cams > thr, 0, cams)
    mean[hw]    = sum_k dropped[k, hw] / 4
    out[n,c,hw] = x[n,c,hw] * mean[hw]

Strategy: data-parallel over batch N=32 across 8 NeuronCores (4 samples
per core).  The kernel is purely DMA-bound (read x once, write out once;
compute is tiny), so all device I/O is float16: x is cast to fp16 on the
host before upload, the kernel computes with f32 accumulation (PE PSUM,
threshold compare in f32) and stores fp16 outputs which the host casts
back to f32.  That halves both DMA directions vs f32 while keeping the
global rel err ~7e-3 (fp16 product noise is small enough that the
gama*max threshold compare flips essentially no mask bits).

Per sample, x[n] ([4096, 784] fp16, 6.4 MB) is loaded into SBUF once as
8 piece tiles [128, 4, 784] and kept resident: the 1x1 conv runs as 32
accumulating fp16 matmuls (contraction 4096 = 32x128 on partitions)
into PSUM [4, 784] f32; the per-channel max / threshold / mask run on
ACT+DVE in f32; the channel-mean + broadcast to 128 partitions is a
single matmul with a constant [4, 128] lhsT of 0.25, converted to an
fp16 SBUF tile on ACT; and the final elementwise multiply (all-fp16 on
DVE, 2x throughput) reuses the resident x tiles in-place before the
store.  HBM traffic is the 2x fp16 floor.
"""

import hashlib
import os
import sys

for _p in ("/opt/trn_rl_repo",):
    if _p not in sys.path:
        sys.path.insert(0, _p)

import numpy as np

N_CORES = 8
NFULL = 32            # full batch
NS = NFULL // N_CORES  # samples per core
C = 4096
K = 4
HW = 28 * 28          # 784
NCHUNK = C // 128     # 32
NQ = 4                # quarter tiles per sample
CPQ = NCHUNK // NQ    # 8 chunks per quarter
HALVES = ((0, 512), (512, HW))  # PSUM-bank-aligned column split

_CACHE = {}


def build_nc(n_pieces=8, x_bufs=17, cams_bufs=2, mean_bufs=2,
             store_engine="scalar", gpsimd_pieces=1, pe_filler=4,
             io_dtype="float16", prefetch=2, last_pieces=16,
             gpsimd_last=3):
    """Trace + schedule + compile the per-core Bass program.

    n_pieces: how many SBUF tiles one sample's x is split into (must
        divide 32); x_bufs slots of [128, 32/n_pieces, 784] each.
    store_engine: which engine issues output DMAs ("sync"/"scalar"/"gpsimd")
        — separate HWDGE ring from the loads avoids FIFO coupling.
    gpsimd_pieces: how many of the per-sample multiply pieces run on
        GpSimd instead of DVE (load balancing).
    io_dtype: dtype of x/w/out device tensors ("float16"/"bfloat16"/
        "float32").  Accumulation and the threshold compare stay f32.
    last_pieces/gpsimd_last: the LAST sample's multiply+store chain is the
        exposed kernel tail (nothing overlaps it), so it gets finer pieces
        and a bigger GpSimd share: multiply stream ~11 us instead of ~14.
    """
    from contextlib import ExitStack

    import concourse.bacc as bacc
    import concourse.tile as tile
    from concourse import mybir

    f32 = mybir.dt.float32
    fio = getattr(mybir.dt, io_dtype)
    nc = bacc.Bacc("TRN2", target_bir_lowering=False, debug=False,
                   num_devices=N_CORES)

    NP = n_pieces
    CPP = NCHUNK // NP  # chunks per piece
    NPL = last_pieces or NP
    CPPL = NCHUNK // NPL

    def fine(n):
        # Sample 0 (head: first store gates the whole store stream) and
        # the last sample (tail: nothing overlaps it) get fine pieces.
        return n == 0 or n == NS - 1

    def sample_np_cpp(n):
        return (NPL, CPPL) if fine(n) else (NP, CPP)

    x_d = nc.dram_tensor("x", [NS, C, HW], fio, kind="ExternalInput")
    w_d = nc.dram_tensor("w", [128, NCHUNK, K], fio, kind="ExternalInput")
    gam_d = nc.dram_tensor("gam", [K, 1], f32, kind="ExternalInput")
    qlhs_d = nc.dram_tensor("qlhs", [K, 128], fio, kind="ExternalInput")
    out_d = nc.dram_tensor("out", [NS, C, HW], fio, kind="ExternalOutput")

    # [NS, C, HW] viewed as [NS, 128(part), NCHUNK, HW]: partition p holds
    # the NCHUNK *adjacent* channels c = p*NCHUNK + j.  Each (partition,
    # piece) DMA run is then CPP*3136 contiguous bytes — fewer, longer
    # descriptors than the chunk-major c = j*128 + p mapping — and the w
    # host packing in make_in_maps is a plain reshape with the same mapping.
    x_v = x_d.ap().rearrange("n (p j) hw -> n p j hw", p=128, j=NCHUNK)
    out_v = out_d.ap().rearrange("n (p j) hw -> n p j hw", p=128, j=NCHUNK)

    if isinstance(store_engine, (list, tuple)):
        store_engs = [getattr(nc, e) for e in store_engine]
    else:
        store_engs = [getattr(nc, store_engine)]

    with tile.TileContext(nc) as tc, ExitStack() as ctx:
        consts = ctx.enter_context(tc.tile_pool(name="consts", bufs=1))
        xpool = ctx.enter_context(tc.tile_pool(name="xpool", bufs=x_bufs))
        xlpool = ctx.enter_context(tc.tile_pool(name="xlpool", bufs=NPL))
        spool = ctx.enter_context(tc.tile_pool(name="spool", bufs=2))
        cpsum = ctx.enter_context(
            tc.tile_pool(name="cpsum", bufs=cams_bufs, space="PSUM"))
        mpsum = ctx.enter_context(
            tc.tile_pool(name="mpsum", bufs=mean_bufs, space="PSUM"))

        w_sb = consts.tile([128, NCHUNK, K], fio, name="w_sb")
        nc.scalar.dma_start(w_sb[:], w_d.ap())
        gam_sb = consts.tile([K, 1], f32, name="gam_sb")
        nc.scalar.dma_start(gam_sb[:], gam_d.ap())
        qlhs_sb = consts.tile([K, 128], fio, name="qlhs_sb")
        nc.scalar.dma_start(qlhs_sb[:], qlhs_d.ap())

        xq_all = {}
        cams_all = {}

        def emit_loads(n, engines=(nc.sync,)):
            # Sample 0 may split across both HWDGE rings: the store ring
            # (qActDynamicHW) is empty until the first mean is ready, so
            # lending it to the head loads gets mean(0) (and with it the
            # whole store stream) started earlier.  Later samples must stay
            # off the store ring: a load enqueued behind a not-yet-ready
            # store would head-of-line block there.
            np_, cpp_ = sample_np_cpp(n)
            # Fine samples (0 and NS-1) share xlpool slots: sample 0's
            # tiles are stored (and freed) long before the last sample's
            # loads want the slots back.
            pool = xlpool if fine(n) else xpool
            xq_all[n] = []
            for q in range(np_):
                t = pool.tile([128, cpp_, HW], fio, tag="xq",
                              name=f"xq_{n}_{q}")
                engines[q % len(engines)].dma_start(
                    t[:], x_v[n][:, q * cpp_:(q + 1) * cpp_, :])
                xq_all[n].append(t)

        def emit_chunk_mms(n, j_lo, j_hi):
            cams = cams_all[n]
            xq = xq_all[n]
            _, cpp_ = sample_np_cpp(n)
            for j in range(j_lo, j_hi):
                q, jj = divmod(j, cpp_)
                for c0, c1 in HALVES:
                    nc.tensor.matmul(
                        cams[:, c0:c1],
                        w_sb[:, j, :],
                        xq[q][:, jj, c0:c1],
                        start=(j == 0),
                        stop=(j == NCHUNK - 1),
                    )

        # Queue loads `prefetch` samples ahead so the HBM read stream never
        # starves while a sample's mean chain (mask -> mean -> multiply)
        # briefly stalls the consumers.  SBUF holds prefetch+1 samples of x.
        for n0 in range(min(prefetch, NS)):
            emit_loads(n0, engines=(nc.sync, nc.scalar) if n0 == 0
                       else (nc.sync,))
        for n in range(NS):
            if n not in cams_all:
                cams_all[n] = cpsum.tile([K, HW], f32, tag="cams",
                                         name=f"cams_{n}")
            emit_chunk_mms(n, pe_filler if n > 0 else 0, NCHUNK)
            cams = cams_all[n]
            xq = xq_all[n]

            # relu on ACT (PSUM -> SBUF), concurrent with DVE's max below —
            # both only read the completed cams PSUM tile.
            r = spool.tile([K, HW], f32, tag="r", name=f"r_{n}")
            nc.scalar.activation(r[:], cams[:],
                                 mybir.ActivationFunctionType.Relu)
            # per-channel spatial max straight from PSUM (pre-relu):
            # max(relu(cams)) == max(0, max(cams)), folded into thr below.
            # Overlapping relu and the reduce shortens the serial
            # mask-chain (it gates each sample's store burst and the tail).
            rmax = spool.tile([K, 1], f32, tag="rmax", name=f"rmax_{n}")
            nc.vector.tensor_reduce(rmax[:], cams[:], axis=mybir.AxisListType.X,
                                    op=mybir.AluOpType.max)
            # thr = gama * max(0, rmax)
            thr = spool.tile([K, 1], f32, tag="thr", name=f"thr_{n}")
            nc.vector.tensor_scalar(thr[:], rmax[:], 0.0, gam_sb[:],
                                    op0=mybir.AluOpType.max,
                                    op1=mybir.AluOpType.mult)
            # masked = (r <= thr) * r, cast to the IO dtype for the mean
            # matmul (f32 compare, so no extra mask flips from this cast)
            masked = spool.tile([K, HW], fio, tag="masked", name=f"masked_{n}")
            nc.vector.scalar_tensor_tensor(masked[:], r[:], thr[:], r[:],
                                           op0=mybir.AluOpType.is_le,
                                           op1=mybir.AluOpType.mult)
            # Keep PE busy while the DVE mask for sample n completes:
            # emit the first pe_filler chunk matmuls of sample n+1 ahead of
            # sample n's mean matmul in PE program order (in-order engine,
            # head-of-line blocking otherwise; also avoids a HAM idle gap).
            if n + prefetch < NS:
                emit_loads(n + prefetch)
            if n + 1 < NS and pe_filler:
                cams_all[n + 1] = cpsum.tile([K, HW], f32, tag="cams",
                                             name=f"cams_{n + 1}")
                emit_chunk_mms(n + 1, 0, pe_filler)

            # mean over k, broadcast to 128 partitions: qlhs (0.25) matmul
            meanb = mpsum.tile([128, HW], f32, tag="meanb", name=f"meanb_{n}")
            for c0, c1 in HALVES:
                nc.tensor.matmul(meanb[:, c0:c1], qlhs_sb[:],
                                 masked[:, c0:c1], start=True, stop=True)

            # PSUM f32 -> SBUF fio on ACT so the final multiply is all
            # 16-bit on DVE (2x throughput) and PSUM banks free up early.
            mean_sb = spool.tile([128, HW], fio, tag="mean_sb",
                                 name=f"mean_sb_{n}")
            nc.scalar.activation(mean_sb[:], meanb[:],
                                 mybir.ActivationFunctionType.Copy)

            # For the last sample all loads are already enqueued, so the
            # load ring (qSPDynamicHW) is free to take half the tail
            # stores — both rings drain the final store backlog in
            # parallel.  (Mirror of the sample-0 load split above.)
            last = n == NS - 1
            s_engs = (store_engs + [nc.sync]) if last else store_engs
            np_, cpp_ = sample_np_cpp(n)
            gps = gpsimd_last if fine(n) else gpsimd_pieces
            mb = mean_sb.unsqueeze(1).broadcast_to([128, cpp_, HW])
            for q in range(np_):
                # GpSimd is ~3.7x slower per piece than DVE; give it the
                # LAST pieces so its slow multiplies never head-of-line
                # block the store FIFO in front of fast DVE pieces (it
                # starts at mean-ready regardless, trimming DVE's chain).
                eng = nc.gpsimd if q >= np_ - gps else nc.vector
                eng.tensor_tensor(xq[q][:], xq[q][:], mb,
                                  op=mybir.AluOpType.mult)
                s_engs[q % len(s_engs)].dma_start(
                    out_v[n][:, q * cpp_:(q + 1) * cpp_, :], xq[q][:])

    nc.compile()
    return nc


def _get_nc():
    if "nc" not in _CACHE:
        _CACHE["nc"] = build_nc()
    return _CACHE["nc"]


IO_NP_DTYPE = np.float16


def make_in_maps(x, fc_weights, gama):
    """Shard/pack full numpy inputs into per-core input maps."""
    x = np.ascontiguousarray(
        np.asarray(x, dtype=np.float32).reshape(NFULL, C, HW).astype(IO_NP_DTYPE))
    fcw = np.asarray(fc_weights, dtype=np.float32).reshape(K, C)
    # w_arr[p, j, k] = fcw[k, p*NCHUNK + j]  (channel c = p*NCHUNK + j,
    # matching the x view in build_nc)
    w_arr = np.ascontiguousarray(fcw.T.reshape(128, NCHUNK, K)).astype(IO_NP_DTYPE)
    gam4 = np.full((K, 1), np.float32(np.asarray(gama)), dtype=np.float32)
    qlhs = np.full((K, 128), 0.25, dtype=IO_NP_DTYPE)
    in_maps = []
    for c in range(N_CORES):
        in_maps.append({
            "x": x[c * NS:(c + 1) * NS],
            "w": w_arr,
            "gam": gam4,
            "qlhs": qlhs,
        })
    return in_maps


def _strip_debug(obj):
    """Recursively blank debug-only fields (file paths / tracebacks) so the
    cache key is independent of where kernel.py lives on disk."""
    if isinstance(obj, dict):
        return {
            k: ("" if k in ("filename", "ant_traceback") else _strip_debug(v))
            for k, v in obj.items()
        }
    if isinstance(obj, list):
        return [_strip_debug(v) for v in obj]
    return obj


def _bass_module_cache_key(code, code_format):
    """Semantic cache key for a bass_exec HLO module, or None.

    Hashes the embedded BIR with debug-only fields blanked, plus the
    IO-name/arch config.  Any semantic difference changes the key; a
    path-only difference (same kernel traced from another directory)
    does not.
    """
    import base64
    import json

    if b"bass_exec" not in bytes(code) or bytes(code_format) != b"hlo":
        return None
    import libneuronxla.proto.hlo_pb2 as hlo_pb2
    from concourse import bass2jax

    proto = hlo_pb2.HloModuleProto.FromString(bytes(code))
    cfgs = [
        ins.backend_config
        for comp in proto.computations
        for ins in comp.instructions
        if ins.opcode == "custom-call" and ins.custom_call_target == "bass_exec"
    ]
    if len(cfgs) != 1:
        return None
    config = json.loads(base64.standard_b64decode(cfgs[0]))
    decomp = getattr(bass2jax, "_decompress_ant_bir", None)
    if decomp is None:
        return None
    bir = json.loads(decomp(config["ant_bir"]))
    h = hashlib.sha256()
    h.update(json.dumps(_strip_debug(bir), sort_keys=True).encode())
    h.update(json.dumps(
        [config.get("in_names"), config.get("out_names"),
         config.get("arch"), proto.name],
        sort_keys=True).encode())
    return h.hexdigest()


def _install_neff_cache():
    """Wrap concourse's neuronx_cc hook with a content-keyed NEFF cache.

    The stock hook recompiles the NEFF from scratch in every process
    (minutes for this kernel); the emitted BIR is deterministic modulo
    debug file paths, so a debug-stripped content hash makes repeat
    compiles of the identical module instant.
    """
    if _CACHE.get("cc_cached"):
        return
    try:
        from concourse import bass2jax

        inner = bass2jax.neuronx_cc_hook
        cache_dir = os.path.expanduser("~/.cache/bass_neff_cache")
        os.makedirs(cache_dir, exist_ok=True)

        def cached_hook(code, code_format, platform_version, file_prefix):
            path = None
            try:
                key = _bass_module_cache_key(code, code_format)
                if key is not None:
                    path = os.path.join(cache_dir, key)
                    if os.path.exists(path):
                        with open(path, "rb") as f:
                            return 0, f.read()
            except Exception:
                path = None
            ret, data = inner(code, code_format, platform_version, file_prefix)
            if path is not None and ret == 0:
                try:
                    tmp = f"{path}.tmp{os.getpid()}"
                    with open(tmp, "wb") as f:
                        f.write(data)
                    os.replace(tmp, path)
                except Exception:
                    pass
            return ret, data

        bass2jax.neuronx_cc_hook = cached_hook
        # If the plain hook was already installed on libneuronxla, refresh it.
        try:
            import libneuronxla

            if getattr(libneuronxla, "orig_neuronx_cc", None) is not None:
                libneuronxla.neuronx_cc = cached_hook
        except ImportError:
            pass
        _CACHE["cc_cached"] = True
    except Exception:
        pass


def kernel(x, fc_weights, gama):
    from concourse.bass_utils import run_bass_kernel_spmd

    _install_neff_cache()
    nc = _get_nc()
    in_maps = make_in_maps(x, fc_weights, gama)
    res = run_bass_kernel_spmd(nc, in_maps, core_ids=list(range(N_CORES)))
    out = np.concatenate([np.asarray(r["out"]) for r in res.results], axis=0)
    return out.reshape(NFULL, C, 28, 28).astype(np.float32)



# revision 29
# speedup vs baseline: 1.4784x; 1.0030x over previous
"""Trainium2 Bass kernel for the topk-masking attention module.

Computation (per sample n):
    cams[k, hw] = relu(sum_c x[n, c, hw] * w[k, c])          # 1x1 conv, K=4
    thr[k]      = gama * max_hw(cams[k, :])
    dropped     = where(cams > thr, 0, cams)
    mean[hw]    = sum_k dropped[k, hw] / 4
    out[n,c,hw] = x[n,c,hw] * mean[hw]

Strategy: data-parallel over batch N=32 across 8 NeuronCores (4 samples
per core).  The kernel is purely DMA-bound (read x once, write out once;
compute is tiny), so all device I/O is float16: x is cast to fp16 on the
host before upload, the kernel computes with f32 accumulation (PE PSUM,
threshold compare in f32) and stores fp16 outputs which the host casts
back to f32.  That halves both DMA directions vs f32 while keeping the
global rel err ~7e-3 (fp16 product noise is small enough that the
gama*max threshold compare flips essentially no mask bits).

Per sample, x[n] ([4096, 784] fp16, 6.4 MB) is loaded into SBUF once as
8 piece tiles [128, 4, 784] and kept resident: the 1x1 conv runs as 32
accumulating fp16 matmuls (contraction 4096 = 32x128 on partitions)
into PSUM [4, 784] f32; the per-channel max / threshold / mask run on
ACT+DVE in f32; the channel-mean + broadcast to 128 partitions is a
single matmul with a constant [4, 128] lhsT of 0.25, converted to an
fp16 SBUF tile on ACT; and the final elementwise multiply (all-fp16 on
DVE, 2x throughput) reuses the resident x tiles in-place before the
store.  HBM traffic is the 2x fp16 floor.
"""

import hashlib
import os
import sys

for _p in ("/opt/trn_rl_repo",):
    if _p not in sys.path:
        sys.path.insert(0, _p)

import numpy as np

N_CORES = 8
NFULL = 32            # full batch
NS = NFULL // N_CORES  # samples per core
C = 4096
K = 4
HW = 28 * 28          # 784
NCHUNK = C // 128     # 32
NQ = 4                # quarter tiles per sample
CPQ = NCHUNK // NQ    # 8 chunks per quarter
HALVES = ((0, 512), (512, HW))  # PSUM-bank-aligned column split

_CACHE = {}


def build_nc(n_pieces=8, x_bufs=17, cams_bufs=2, mean_bufs=2,
             store_engine="scalar", gpsimd_pieces=1, pe_filler=4,
             io_dtype="float16", prefetch=2, last_pieces=16,
             gpsimd_last=3):
    """Trace + schedule + compile the per-core Bass program.

    n_pieces: how many SBUF tiles one sample's x is split into (must
        divide 32); x_bufs slots of [128, 32/n_pieces, 784] each.
    store_engine: which engine issues output DMAs ("sync"/"scalar"/"gpsimd")
        — separate HWDGE ring from the loads avoids FIFO coupling.
    gpsimd_pieces: how many of the per-sample multiply pieces run on
        GpSimd instead of DVE (load balancing).
    io_dtype: dtype of x/w/out device tensors ("float16"/"bfloat16"/
        "float32").  Accumulation and the threshold compare stay f32.
    last_pieces/gpsimd_last: the LAST sample's multiply+store chain is the
        exposed kernel tail (nothing overlaps it), so it gets finer pieces
        and a bigger GpSimd share: multiply stream ~11 us instead of ~14.
    """
    from contextlib import ExitStack

    import concourse.bacc as bacc
    import concourse.tile as tile
    from concourse import mybir

    f32 = mybir.dt.float32
    fio = getattr(mybir.dt, io_dtype)
    nc = bacc.Bacc("TRN2", target_bir_lowering=False, debug=False,
                   num_devices=N_CORES)

    NP = n_pieces
    CPP = NCHUNK // NP  # chunks per piece
    NPL = last_pieces or NP
    CPPL = NCHUNK // NPL

    def fine(n):
        # Sample 0 (head: first store gates the whole store stream) and
        # the last sample (tail: nothing overlaps it) get fine pieces.
        return n == 0 or n == NS - 1

    def sample_np_cpp(n):
        return (NPL, CPPL) if fine(n) else (NP, CPP)

    x_d = nc.dram_tensor("x", [NS, C, HW], fio, kind="ExternalInput")
    w_d = nc.dram_tensor("w", [128, NCHUNK, K], fio, kind="ExternalInput")
    gam_d = nc.dram_tensor("gam", [K, 1], f32, kind="ExternalInput")
    qlhs_d = nc.dram_tensor("qlhs", [K, 128], fio, kind="ExternalInput")
    out_d = nc.dram_tensor("out", [NS, C, HW], fio, kind="ExternalOutput")

    # [NS, C, HW] viewed as [NS, 128(part), NCHUNK, HW]: partition p holds
    # the NCHUNK *adjacent* channels c = p*NCHUNK + j.  Each (partition,
    # piece) DMA run is then CPP*3136 contiguous bytes — fewer, longer
    # descriptors than the chunk-major c = j*128 + p mapping — and the w
    # host packing in make_in_maps is a plain reshape with the same mapping.
    x_v = x_d.ap().rearrange("n (p j) hw -> n p j hw", p=128, j=NCHUNK)
    out_v = out_d.ap().rearrange("n (p j) hw -> n p j hw", p=128, j=NCHUNK)

    if isinstance(store_engine, (list, tuple)):
        store_engs = [getattr(nc, e) for e in store_engine]
    else:
        store_engs = [getattr(nc, store_engine)]

    with tile.TileContext(nc) as tc, ExitStack() as ctx:
        consts = ctx.enter_context(tc.tile_pool(name="consts", bufs=1))
        xpool = ctx.enter_context(tc.tile_pool(name="xpool", bufs=x_bufs))
        xlpool = ctx.enter_context(tc.tile_pool(name="xlpool", bufs=NPL))
        spool = ctx.enter_context(tc.tile_pool(name="spool", bufs=2))
        cpsum = ctx.enter_context(
            tc.tile_pool(name="cpsum", bufs=cams_bufs, space="PSUM"))
        mpsum = ctx.enter_context(
            tc.tile_pool(name="mpsum", bufs=mean_bufs, space="PSUM"))

        w_sb = consts.tile([128, NCHUNK, K], fio, name="w_sb")
        nc.scalar.dma_start(w_sb[:], w_d.ap())
        gam_sb = consts.tile([K, 1], f32, name="gam_sb")
        nc.scalar.dma_start(gam_sb[:], gam_d.ap())
        qlhs_sb = consts.tile([K, 128], fio, name="qlhs_sb")
        nc.scalar.dma_start(qlhs_sb[:], qlhs_d.ap())

        xq_all = {}
        cams_all = {}

        def emit_loads(n, engines=(nc.sync,)):
            # Sample 0 may split across both HWDGE rings: the store ring
            # (qActDynamicHW) is empty until the first mean is ready, so
            # lending it to the head loads gets mean(0) (and with it the
            # whole store stream) started earlier.  Later samples must stay
            # off the store ring: a load enqueued behind a not-yet-ready
            # store would head-of-line block there.
            np_, cpp_ = sample_np_cpp(n)
            # Fine samples (0 and NS-1) share xlpool slots: sample 0's
            # tiles are stored (and freed) long before the last sample's
            # loads want the slots back.
            pool = xlpool if fine(n) else xpool
            xq_all[n] = []
            for q in range(np_):
                t = pool.tile([128, cpp_, HW], fio, tag="xq",
                              name=f"xq_{n}_{q}")
                engines[q % len(engines)].dma_start(
                    t[:], x_v[n][:, q * cpp_:(q + 1) * cpp_, :])
                xq_all[n].append(t)

        def emit_chunk_mms(n, j_lo, j_hi):
            cams = cams_all[n]
            xq = xq_all[n]
            _, cpp_ = sample_np_cpp(n)
            for j in range(j_lo, j_hi):
                q, jj = divmod(j, cpp_)
                for c0, c1 in HALVES:
                    nc.tensor.matmul(
                        cams[:, c0:c1],
                        w_sb[:, j, :],
                        xq[q][:, jj, c0:c1],
                        start=(j == 0),
                        stop=(j == NCHUNK - 1),
                    )

        # Queue loads `prefetch` samples ahead so the HBM read stream never
        # starves while a sample's mean chain (mask -> mean -> multiply)
        # briefly stalls the consumers.  SBUF holds prefetch+1 samples of x.
        for n0 in range(min(prefetch, NS)):
            emit_loads(n0, engines=(nc.sync, nc.scalar) if n0 == 0
                       else (nc.sync,))
        for n in range(NS):
            if n not in cams_all:
                cams_all[n] = cpsum.tile([K, HW], f32, tag="cams",
                                         name=f"cams_{n}")
            emit_chunk_mms(n, pe_filler if n > 0 else 0, NCHUNK)
            cams = cams_all[n]
            xq = xq_all[n]

            # relu on ACT (PSUM -> SBUF), concurrent with DVE's max below —
            # both only read the completed cams PSUM tile.
            r = spool.tile([K, HW], f32, tag="r", name=f"r_{n}")
            nc.scalar.activation(r[:], cams[:],
                                 mybir.ActivationFunctionType.Relu)
            # per-channel spatial max straight from PSUM (pre-relu):
            # max(relu(cams)) == max(0, max(cams)), folded into thr below.
            # Overlapping relu and the reduce shortens the serial
            # mask-chain (it gates each sample's store burst and the tail).
            rmax = spool.tile([K, 1], f32, tag="rmax", name=f"rmax_{n}")
            nc.vector.tensor_reduce(rmax[:], cams[:], axis=mybir.AxisListType.X,
                                    op=mybir.AluOpType.max)
            # thr = gama * max(0, rmax)
            thr = spool.tile([K, 1], f32, tag="thr", name=f"thr_{n}")
            nc.vector.tensor_scalar(thr[:], rmax[:], 0.0, gam_sb[:],
                                    op0=mybir.AluOpType.max,
                                    op1=mybir.AluOpType.mult)
            # masked = (r <= thr) * r, cast to the IO dtype for the mean
            # matmul (f32 compare, so no extra mask flips from this cast)
            masked = spool.tile([K, HW], fio, tag="masked", name=f"masked_{n}")
            nc.vector.scalar_tensor_tensor(masked[:], r[:], thr[:], r[:],
                                           op0=mybir.AluOpType.is_le,
                                           op1=mybir.AluOpType.mult)
            # Keep PE busy while the DVE mask for sample n completes:
            # emit the first pe_filler chunk matmuls of sample n+1 ahead of
            # sample n's mean matmul in PE program order (in-order engine,
            # head-of-line blocking otherwise; also avoids a HAM idle gap).
            if n + prefetch < NS:
                emit_loads(n + prefetch)
            if n + 1 < NS and pe_filler:
                cams_all[n + 1] = cpsum.tile([K, HW], f32, tag="cams",
                                             name=f"cams_{n + 1}")
                emit_chunk_mms(n + 1, 0, pe_filler)

            # mean over k, broadcast to 128 partitions: qlhs (0.25) matmul
            meanb = mpsum.tile([128, HW], f32, tag="meanb", name=f"meanb_{n}")
            for c0, c1 in HALVES:
                nc.tensor.matmul(meanb[:, c0:c1], qlhs_sb[:],
                                 masked[:, c0:c1], start=True, stop=True)

            # PSUM f32 -> SBUF fio on ACT so the final multiply is all
            # 16-bit on DVE (2x throughput) and PSUM banks free up early.
            mean_sb = spool.tile([128, HW], fio, tag="mean_sb",
                                 name=f"mean_sb_{n}")
            nc.scalar.activation(mean_sb[:], meanb[:],
                                 mybir.ActivationFunctionType.Copy)

            # For the last TWO samples all loads are already enqueued in
            # program order (s3's loads are emitted during s1's block), so
            # the load ring (qSPDynamicHW) can take half their stores with
            # no head-of-line risk — this rebalances ring bytes (the store
            # ring also carried half of sample 0's loads) and drains the
            # final store backlog on both rings in parallel.
            last = n == NS - 1
            s_engs = (store_engs + [nc.sync]) if n >= NS - 2 else store_engs
            np_, cpp_ = sample_np_cpp(n)
            gps = gpsimd_last if fine(n) else gpsimd_pieces
            mb = mean_sb.unsqueeze(1).broadcast_to([128, cpp_, HW])
            for q in range(np_):
                # GpSimd is ~3.7x slower per piece than DVE; give it the
                # LAST pieces so its slow multiplies never head-of-line
                # block the store FIFO in front of fast DVE pieces (it
                # starts at mean-ready regardless, trimming DVE's chain).
                eng = nc.gpsimd if q >= np_ - gps else nc.vector
                eng.tensor_tensor(xq[q][:], xq[q][:], mb,
                                  op=mybir.AluOpType.mult)
                s_engs[q % len(s_engs)].dma_start(
                    out_v[n][:, q * cpp_:(q + 1) * cpp_, :], xq[q][:])

    nc.compile()
    return nc


def _get_nc():
    if "nc" not in _CACHE:
        _CACHE["nc"] = build_nc()
    return _CACHE["nc"]


IO_NP_DTYPE = np.float16


def make_in_maps(x, fc_weights, gama):
    """Shard/pack full numpy inputs into per-core input maps."""
    x = np.ascontiguousarray(
        np.asarray(x, dtype=np.float32).reshape(NFULL, C, HW).astype(IO_NP_DTYPE))
    fcw = np.asarray(fc_weights, dtype=np.float32).reshape(K, C)
    # w_arr[p, j, k] = fcw[k, p*NCHUNK + j]  (channel c = p*NCHUNK + j,
    # matching the x view in build_nc)
    w_arr = np.ascontiguousarray(fcw.T.reshape(128, NCHUNK, K)).astype(IO_NP_DTYPE)
    gam4 = np.full((K, 1), np.float32(np.asarray(gama)), dtype=np.float32)
    qlhs = np.full((K, 128), 0.25, dtype=IO_NP_DTYPE)
    in_maps = []
    for c in range(N_CORES):
        in_maps.append({
            "x": x[c * NS:(c + 1) * NS],
            "w": w_arr,
            "gam": gam4,
            "qlhs": qlhs,
        })
    return in_maps


def _strip_debug(obj):
    """Recursively blank debug-only fields (file paths / tracebacks) so the
    cache key is independent of where kernel.py lives on disk."""
    if isinstance(obj, dict):
        return {
            k: ("" if k in ("filename", "ant_traceback") else _strip_debug(v))
            for k, v in obj.items()
        }
    if isinstance(obj, list):
        return [_strip_debug(v) for v in obj]
    return obj


def _bass_module_cache_key(code, code_format):
    """Semantic cache key for a bass_exec HLO module, or None.

    Hashes the embedded BIR with debug-only fields blanked, plus the
    IO-name/arch config.  Any semantic difference changes the key; a
    path-only difference (same kernel traced from another directory)
    does not.
    """
    import base64
    import json

    if b"bass_exec" not in bytes(code) or bytes(code_format) != b"hlo":
        return None
    import libneuronxla.proto.hlo_pb2 as hlo_pb2
    from concourse import bass2jax

    proto = hlo_pb2.HloModuleProto.FromString(bytes(code))
    cfgs = [
        ins.backend_config
        for comp in proto.computations
        for ins in comp.instructions
        if ins.opcode == "custom-call" and ins.custom_call_target == "bass_exec"
    ]
    if len(cfgs) != 1:
        return None
    config = json.loads(base64.standard_b64decode(cfgs[0]))
    decomp = getattr(bass2jax, "_decompress_ant_bir", None)
    if decomp is None:
        return None
    bir = json.loads(decomp(config["ant_bir"]))
    h = hashlib.sha256()
    h.update(json.dumps(_strip_debug(bir), sort_keys=True).encode())
    h.update(json.dumps(
        [config.get("in_names"), config.get("out_names"),
         config.get("arch"), proto.name],
        sort_keys=True).encode())
    return h.hexdigest()


def _install_neff_cache():
    """Wrap concourse's neuronx_cc hook with a content-keyed NEFF cache.

    The stock hook recompiles the NEFF from scratch in every process
    (minutes for this kernel); the emitted BIR is deterministic modulo
    debug file paths, so a debug-stripped content hash makes repeat
    compiles of the identical module instant.
    """
    if _CACHE.get("cc_cached"):
        return
    try:
        from concourse import bass2jax

        inner = bass2jax.neuronx_cc_hook
        cache_dir = os.path.expanduser("~/.cache/bass_neff_cache")
        os.makedirs(cache_dir, exist_ok=True)

        def cached_hook(code, code_format, platform_version, file_prefix):
            path = None
            try:
                key = _bass_module_cache_key(code, code_format)
                if key is not None:
                    path = os.path.join(cache_dir, key)
                    if os.path.exists(path):
                        with open(path, "rb") as f:
                            return 0, f.read()
            except Exception:
                path = None
            ret, data = inner(code, code_format, platform_version, file_prefix)
            if path is not None and ret == 0:
                try:
                    tmp = f"{path}.tmp{os.getpid()}"
                    with open(tmp, "wb") as f:
                        f.write(data)
                    os.replace(tmp, path)
                except Exception:
                    pass
            return ret, data

        bass2jax.neuronx_cc_hook = cached_hook
        # If the plain hook was already installed on libneuronxla, refresh it.
        try:
            import libneuronxla

            if getattr(libneuronxla, "orig_neuronx_cc", None) is not None:
                libneuronxla.neuronx_cc = cached_hook
        except ImportError:
            pass
        _CACHE["cc_cached"] = True
    except Exception:
        pass


def kernel(x, fc_weights, gama):
    from concourse.bass_utils import run_bass_kernel_spmd

    _install_neff_cache()
    nc = _get_nc()
    in_maps = make_in_maps(x, fc_weights, gama)
    res = run_bass_kernel_spmd(nc, in_maps, core_ids=list(range(N_CORES)))
    out = np.concatenate([np.asarray(r["out"]) for r in res.results], axis=0)
    return out.reshape(NFULL, C, 28, 28).astype(np.float32)



# revision 30
# speedup vs baseline: 1.7513x; 1.1846x over previous
"""Trainium2 Bass kernel for the topk-masking attention module.

Computation (per sample n):
    cams[k, hw] = relu(sum_c x[n, c, hw] * w[k, c])          # 1x1 conv, K=4
    thr[k]      = gama * max_hw(cams[k, :])
    dropped     = where(cams > thr, 0, cams)
    mean[hw]    = sum_k dropped[k, hw] / 4
    out[n,c,hw] = x[n,c,hw] * mean[hw]

Strategy: data-parallel over batch N=32 across 8 NeuronCores (4 samples
per core).  The kernel is purely DMA-bound (read x once, write out once;
compute is tiny), so all device I/O is float16: x is cast to fp16 on the
host before upload, the kernel computes with f32 accumulation (PE PSUM,
threshold compare in f32) and stores fp16 outputs which the host casts
back to f32.  That halves both DMA directions vs f32 while keeping the
global rel err ~7e-3 (fp16 product noise is small enough that the
gama*max threshold compare flips essentially no mask bits).

Per sample, x[n] ([4096, 784] fp16, 6.4 MB) is loaded into SBUF once as
8 piece tiles [128, 4, 784] and kept resident: the 1x1 conv runs as 32
accumulating fp16 matmuls (contraction 4096 = 32x128 on partitions)
into PSUM [4, 784] f32; the per-channel max / threshold / mask run on
ACT+DVE in f32; the channel-mean + broadcast to 128 partitions is a
single matmul with a constant [4, 128] lhsT of 0.25, converted to an
fp16 SBUF tile on ACT; and the final elementwise multiply (all-fp16 on
DVE, 2x throughput) reuses the resident x tiles in-place before the
store.  HBM traffic is the 2x fp16 floor.
"""

import hashlib
import os
import sys

for _p in ("/opt/trn_rl_repo",):
    if _p not in sys.path:
        sys.path.insert(0, _p)

import numpy as np

N_CORES = 8
NFULL = 32            # full batch
NS = NFULL // N_CORES  # samples per core
C = 4096
K = 4
HW = 28 * 28          # 784
NCHUNK = C // 128     # 32
NQ = 4                # quarter tiles per sample
CPQ = NCHUNK // NQ    # 8 chunks per quarter
HALVES = ((0, 512), (512, HW))  # PSUM-bank-aligned column split

_CACHE = {}


def build_nc(n_pieces=8, x_bufs=17, cams_bufs=2, mean_bufs=2,
             store_engine="scalar", gpsimd_pieces=1, pe_filler=4,
             io_dtype="float16", prefetch=2, last_pieces=16,
             gpsimd_last=3):
    """Trace + schedule + compile the per-core Bass program.

    n_pieces: how many SBUF tiles one sample's x is split into (must
        divide 32); x_bufs slots of [128, 32/n_pieces, 784] each.
    store_engine: which engine issues output DMAs ("sync"/"scalar"/"gpsimd")
        — separate HWDGE ring from the loads avoids FIFO coupling.
    gpsimd_pieces: how many of the per-sample multiply pieces run on
        GpSimd instead of DVE (load balancing).
    io_dtype: dtype of x/w/out device tensors ("float16"/"bfloat16"/
        "float32").  Accumulation and the threshold compare stay f32.
    last_pieces/gpsimd_last: the LAST sample's multiply+store chain is the
        exposed kernel tail (nothing overlaps it), so it gets finer pieces
        and a bigger GpSimd share: multiply stream ~11 us instead of ~14.
    """
    from contextlib import ExitStack

    import concourse.bacc as bacc
    import concourse.tile as tile
    from concourse import mybir

    f32 = mybir.dt.float32
    fio = getattr(mybir.dt, io_dtype)
    nc = bacc.Bacc("TRN2", target_bir_lowering=False, debug=False,
                   num_devices=N_CORES)

    NP = n_pieces
    CPP = NCHUNK // NP  # chunks per piece
    NPL = last_pieces or NP
    CPPL = NCHUNK // NPL

    def fine(n):
        # Sample 0 (head: first store gates the whole store stream) and
        # the last sample (tail: nothing overlaps it) get fine pieces.
        return n == 0 or n == NS - 1

    def sample_np_cpp(n):
        return (NPL, CPPL) if fine(n) else (NP, CPP)

    x_d = nc.dram_tensor("x", [NS, C, HW], fio, kind="ExternalInput")
    w_d = nc.dram_tensor("w", [128, NCHUNK, K], fio, kind="ExternalInput")
    gam_d = nc.dram_tensor("gam", [K, 1], f32, kind="ExternalInput")
    qlhs_d = nc.dram_tensor("qlhs", [K, 128], fio, kind="ExternalInput")
    out_d = nc.dram_tensor("out", [NS, C, HW], fio, kind="ExternalOutput")

    # [NS, C, HW] viewed as [NS, 128(part), NCHUNK, HW]: partition p holds
    # the NCHUNK *adjacent* channels c = p*NCHUNK + j.  Each (partition,
    # piece) DMA run is then CPP*3136 contiguous bytes — fewer, longer
    # descriptors than the chunk-major c = j*128 + p mapping — and the w
    # host packing in make_in_maps is a plain reshape with the same mapping.
    x_v = x_d.ap().rearrange("n (p j) hw -> n p j hw", p=128, j=NCHUNK)
    out_v = out_d.ap().rearrange("n (p j) hw -> n p j hw", p=128, j=NCHUNK)

    if isinstance(store_engine, (list, tuple)):
        store_engs = [getattr(nc, e) for e in store_engine]
    else:
        store_engs = [getattr(nc, store_engine)]

    with tile.TileContext(nc) as tc, ExitStack() as ctx:
        consts = ctx.enter_context(tc.tile_pool(name="consts", bufs=1))
        xpool = ctx.enter_context(tc.tile_pool(name="xpool", bufs=x_bufs))
        xlpool = ctx.enter_context(tc.tile_pool(name="xlpool", bufs=NPL))
        spool = ctx.enter_context(tc.tile_pool(name="spool", bufs=2))
        cpsum = ctx.enter_context(
            tc.tile_pool(name="cpsum", bufs=cams_bufs, space="PSUM"))
        mpsum = ctx.enter_context(
            tc.tile_pool(name="mpsum", bufs=mean_bufs, space="PSUM"))

        w_sb = consts.tile([128, NCHUNK, K], fio, name="w_sb")
        nc.scalar.dma_start(w_sb[:], w_d.ap())
        gam_sb = consts.tile([K, 1], f32, name="gam_sb")
        nc.scalar.dma_start(gam_sb[:], gam_d.ap())
        qlhs_sb = consts.tile([K, 128], fio, name="qlhs_sb")
        nc.scalar.dma_start(qlhs_sb[:], qlhs_d.ap())

        xq_all = {}
        cams_all = {}

        def emit_loads(n, engines=(nc.sync,)):
            # Sample 0 may split across both HWDGE rings: the store ring
            # (qActDynamicHW) is empty until the first mean is ready, so
            # lending it to the head loads gets mean(0) (and with it the
            # whole store stream) started earlier.  Later samples must stay
            # off the store ring: a load enqueued behind a not-yet-ready
            # store would head-of-line block there.
            np_, cpp_ = sample_np_cpp(n)
            # Fine samples (0 and NS-1) share xlpool slots: sample 0's
            # tiles are stored (and freed) long before the last sample's
            # loads want the slots back.
            pool = xlpool if fine(n) else xpool
            xq_all[n] = []
            for q in range(np_):
                t = pool.tile([128, cpp_, HW], fio, tag="xq",
                              name=f"xq_{n}_{q}")
                engines[q % len(engines)].dma_start(
                    t[:], x_v[n][:, q * cpp_:(q + 1) * cpp_, :])
                xq_all[n].append(t)

        def emit_chunk_mms(n, j_lo, j_hi):
            cams = cams_all[n]
            xq = xq_all[n]
            _, cpp_ = sample_np_cpp(n)
            for j in range(j_lo, j_hi):
                q, jj = divmod(j, cpp_)
                for c0, c1 in HALVES:
                    nc.tensor.matmul(
                        cams[:, c0:c1],
                        w_sb[:, j, :],
                        xq[q][:, jj, c0:c1],
                        start=(j == 0),
                        stop=(j == NCHUNK - 1),
                    )

        # Queue loads `prefetch` samples ahead so the HBM read stream never
        # starves while a sample's mean chain (mask -> mean -> multiply)
        # briefly stalls the consumers.  SBUF holds prefetch+1 samples of x.
        # n0==0: split evenly across both rings (store ring is empty until
        # the first mean).  n0==1: lend the store ring 2 of 8 pieces — they
        # fill its remaining idle window (between s0's odd pieces and the
        # first store becoming ready) and cut the load ring's total bytes,
        # which gates the tail 1:1.
        head_engines = {
            0: (nc.sync, nc.scalar),
            1: (nc.sync, nc.sync, nc.sync, nc.scalar),
        }
        for n0 in range(min(prefetch, NS)):
            emit_loads(n0, engines=head_engines.get(n0, (nc.sync,)))
        for n in range(NS):
            if n not in cams_all:
                cams_all[n] = cpsum.tile([K, HW], f32, tag="cams",
                                         name=f"cams_{n}")
            emit_chunk_mms(n, pe_filler if n > 0 else 0, NCHUNK)
            cams = cams_all[n]
            xq = xq_all[n]

            # relu on ACT (PSUM -> SBUF), concurrent with DVE's max below —
            # both only read the completed cams PSUM tile.
            r = spool.tile([K, HW], f32, tag="r", name=f"r_{n}")
            nc.scalar.activation(r[:], cams[:],
                                 mybir.ActivationFunctionType.Relu)
            # per-channel spatial max straight from PSUM (pre-relu):
            # max(relu(cams)) == max(0, max(cams)), folded into thr below.
            # Overlapping relu and the reduce shortens the serial
            # mask-chain (it gates each sample's store burst and the tail).
            rmax = spool.tile([K, 1], f32, tag="rmax", name=f"rmax_{n}")
            nc.vector.tensor_reduce(rmax[:], cams[:], axis=mybir.AxisListType.X,
                                    op=mybir.AluOpType.max)
            # thr = gama * max(0, rmax)
            thr = spool.tile([K, 1], f32, tag="thr", name=f"thr_{n}")
            nc.vector.tensor_scalar(thr[:], rmax[:], 0.0, gam_sb[:],
                                    op0=mybir.AluOpType.max,
                                    op1=mybir.AluOpType.mult)
            # masked = (r <= thr) * r, cast to the IO dtype for the mean
            # matmul (f32 compare, so no extra mask flips from this cast)
            masked = spool.tile([K, HW], fio, tag="masked", name=f"masked_{n}")
            nc.vector.scalar_tensor_tensor(masked[:], r[:], thr[:], r[:],
                                           op0=mybir.AluOpType.is_le,
                                           op1=mybir.AluOpType.mult)
            # Keep PE busy while the DVE mask for sample n completes:
            # emit the first pe_filler chunk matmuls of sample n+1 ahead of
            # sample n's mean matmul in PE program order (in-order engine,
            # head-of-line blocking otherwise; also avoids a HAM idle gap).
            if n + prefetch < NS:
                emit_loads(n + prefetch)
            if n + 1 < NS and pe_filler:
                cams_all[n + 1] = cpsum.tile([K, HW], f32, tag="cams",
                                             name=f"cams_{n + 1}")
                emit_chunk_mms(n + 1, 0, pe_filler)

            # mean over k, broadcast to 128 partitions: qlhs (0.25) matmul
            meanb = mpsum.tile([128, HW], f32, tag="meanb", name=f"meanb_{n}")
            for c0, c1 in HALVES:
                nc.tensor.matmul(meanb[:, c0:c1], qlhs_sb[:],
                                 masked[:, c0:c1], start=True, stop=True)

            # PSUM f32 -> SBUF fio on ACT so the final multiply is all
            # 16-bit on DVE (2x throughput) and PSUM banks free up early.
            mean_sb = spool.tile([128, HW], fio, tag="mean_sb",
                                 name=f"mean_sb_{n}")
            nc.scalar.activation(mean_sb[:], meanb[:],
                                 mybir.ActivationFunctionType.Copy)

            # For the last TWO samples all loads are already enqueued in
            # program order (s3's loads are emitted during s1's block), so
            # the load ring (qSPDynamicHW) can take half their stores with
            # no head-of-line risk — this rebalances ring bytes (the store
            # ring also carried half of sample 0's loads) and drains the
            # final store backlog on both rings in parallel.
            last = n == NS - 1
            s_engs = (store_engs + [nc.sync]) if n >= NS - 2 else store_engs
            np_, cpp_ = sample_np_cpp(n)
            gps = gpsimd_last if fine(n) else gpsimd_pieces
            mb = mean_sb.unsqueeze(1).broadcast_to([128, cpp_, HW])
            for q in range(np_):
                # GpSimd is ~3.7x slower per piece than DVE; give it the
                # LAST pieces so its slow multiplies never head-of-line
                # block the store FIFO in front of fast DVE pieces (it
                # starts at mean-ready regardless, trimming DVE's chain).
                eng = nc.gpsimd if q >= np_ - gps else nc.vector
                eng.tensor_tensor(xq[q][:], xq[q][:], mb,
                                  op=mybir.AluOpType.mult)
                s_engs[q % len(s_engs)].dma_start(
                    out_v[n][:, q * cpp_:(q + 1) * cpp_, :], xq[q][:])

    nc.compile()
    return nc


def _get_nc():
    if "nc" not in _CACHE:
        _CACHE["nc"] = build_nc()
    return _CACHE["nc"]


IO_NP_DTYPE = np.float16


def make_in_maps(x, fc_weights, gama):
    """Shard/pack full numpy inputs into per-core input maps."""
    x = np.ascontiguousarray(
        np.asarray(x, dtype=np.float32).reshape(NFULL, C, HW).astype(IO_NP_DTYPE))
    fcw = np.asarray(fc_weights, dtype=np.float32).reshape(K, C)
    # w_arr[p, j, k] = fcw[k, p*NCHUNK + j]  (channel c = p*NCHUNK + j,
    # matching the x view in build_nc)
    w_arr = np.ascontiguousarray(fcw.T.reshape(128, NCHUNK, K)).astype(IO_NP_DTYPE)
    gam4 = np.full((K, 1), np.float32(np.asarray(gama)), dtype=np.float32)
    qlhs = np.full((K, 128), 0.25, dtype=IO_NP_DTYPE)
    in_maps = []
    for c in range(N_CORES):
        in_maps.append({
            "x": x[c * NS:(c + 1) * NS],
            "w": w_arr,
            "gam": gam4,
            "qlhs": qlhs,
        })
    return in_maps


def _strip_debug(obj):
    """Recursively blank debug-only fields (file paths / tracebacks) so the
    cache key is independent of where kernel.py lives on disk."""
    if isinstance(obj, dict):
        return {
            k: ("" if k in ("filename", "ant_traceback") else _strip_debug(v))
            for k, v in obj.items()
        }
    if isinstance(obj, list):
        return [_strip_debug(v) for v in obj]
    return obj


def _bass_module_cache_key(code, code_format):
    """Semantic cache key for a bass_exec HLO module, or None.

    Hashes the embedded BIR with debug-only fields blanked, plus the
    IO-name/arch config.  Any semantic difference changes the key; a
    path-only difference (same kernel traced from another directory)
    does not.
    """
    import base64
    import json

    if b"bass_exec" not in bytes(code) or bytes(code_format) != b"hlo":
        return None
    import libneuronxla.proto.hlo_pb2 as hlo_pb2
    from concourse import bass2jax

    proto = hlo_pb2.HloModuleProto.FromString(bytes(code))
    cfgs = [
        ins.backend_config
        for comp in proto.computations
        for ins in comp.instructions
        if ins.opcode == "custom-call" and ins.custom_call_target == "bass_exec"
    ]
    if len(cfgs) != 1:
        return None
    config = json.loads(base64.standard_b64decode(cfgs[0]))
    decomp = getattr(bass2jax, "_decompress_ant_bir", None)
    if decomp is None:
        return None
    bir = json.loads(decomp(config["ant_bir"]))
    h = hashlib.sha256()
    h.update(json.dumps(_strip_debug(bir), sort_keys=True).encode())
    h.update(json.dumps(
        [config.get("in_names"), config.get("out_names"),
         config.get("arch"), proto.name],
        sort_keys=True).encode())
    return h.hexdigest()


def _install_neff_cache():
    """Wrap concourse's neuronx_cc hook with a content-keyed NEFF cache.

    The stock hook recompiles the NEFF from scratch in every process
    (minutes for this kernel); the emitted BIR is deterministic modulo
    debug file paths, so a debug-stripped content hash makes repeat
    compiles of the identical module instant.
    """
    if _CACHE.get("cc_cached"):
        return
    try:
        from concourse import bass2jax

        inner = bass2jax.neuronx_cc_hook
        cache_dir = os.path.expanduser("~/.cache/bass_neff_cache")
        os.makedirs(cache_dir, exist_ok=True)

        def cached_hook(code, code_format, platform_version, file_prefix):
            path = None
            try:
                key = _bass_module_cache_key(code, code_format)
                if key is not None:
                    path = os.path.join(cache_dir, key)
                    if os.path.exists(path):
                        with open(path, "rb") as f:
                            return 0, f.read()
            except Exception:
                path = None
            ret, data = inner(code, code_format, platform_version, file_prefix)
            if path is not None and ret == 0:
                try:
                    tmp = f"{path}.tmp{os.getpid()}"
                    with open(tmp, "wb") as f:
                        f.write(data)
                    os.replace(tmp, path)
                except Exception:
                    pass
            return ret, data

        bass2jax.neuronx_cc_hook = cached_hook
        # If the plain hook was already installed on libneuronxla, refresh it.
        try:
            import libneuronxla

            if getattr(libneuronxla, "orig_neuronx_cc", None) is not None:
                libneuronxla.neuronx_cc = cached_hook
        except ImportError:
            pass
        _CACHE["cc_cached"] = True
    except Exception:
        pass


def kernel(x, fc_weights, gama):
    from concourse.bass_utils import run_bass_kernel_spmd

    _install_neff_cache()
    nc = _get_nc()
    in_maps = make_in_maps(x, fc_weights, gama)
    res = run_bass_kernel_spmd(nc, in_maps, core_ids=list(range(N_CORES)))
    out = np.concatenate([np.asarray(r["out"]) for r in res.results], axis=0)
    return out.reshape(NFULL, C, 28, 28).astype(np.float32)

